# revision 1
# baseline (speedup 1.0000x reference)
"""Trainium2 Bass kernel v2: decomposed rel-pos attention via fp8 DoubleRow.

Score factorization (per batch p, head n), computed transposed S^T[k, q] in
ONE fp8 DoubleRow matmul (256-row contraction, 0.5 cycles/row):
      pair0 rows 0:64   K^T[c,k]       x  Q^T[c,q]/8
      pair0 rows 64:96  onehot_h(k)    x  qterm_h^T[j,q]
      pair0 rows 96:128 onehot_w(k)    x  qterm_w^T[j,q]
      pair1 rows 0:32   kterm_h^T[j,k] x  onehot_h(q)
      pair1 rows 32:64  kterm_w^T[j,k] x  onehot_w(q)
      pair1 rows 64:128 zeros
    et = exp(S^T - 4)        (ACT engine is the bottleneck at ~34us;
                              -4 keeps fp8 range safe, cancels in softmax)
    out^T[q, c] = (et^T V) / (et^T 1) + Q    (PV emitted in [q, c] layout:
                              65-wide free dim makes it nearly free on PE;
                              denominator lands per-partition so normalize
                              is reciprocal + 2 vector ops, no transposes)

qterm/kterm come from 32-wide fp8 table matmuls grouped by h/w of q/k into
one [128, 1024] PSUM tile (4 partition strips: qh, qw, kh, kw), then two
copies (DVE q-side, Pool k-side) drop them into the operand tiles.
q-side tables are pre-scaled x8 so the matmuls read Q/8 in place.

PSUM start_tensor_calc zeroes lazily per (partition-range x 2KB bank): only
the first matmul touching a bank (per partition strip) carries start=True;
w-strips straddle both banks so zero-writing armer matmuls arm them.

Per-batch operand tiles keep DMA/copy/read dependencies exact, and S
matmuls are emitted one k-pair ahead of PV so the in-order PE queue never
head-of-line blocks the exp pipeline.

Sharding: head-parallel across the 8 NeuronCores (4 batches x 1 head each).
"""

import os
import sys

import numpy as np

if "/opt/trn_rl_repo" not in sys.path:
    sys.path.insert(0, "/opt/trn_rl_repo")

B, NH, L, C = 4, 8, 1024, 64
NCORES = 8

_CACHED = {}


def _build_nc():
    import concourse.bass as bass  # noqa: F401
    import concourse.tile as tile
    from concourse import bacc, mybir

    f32 = mybir.dt.float32
    fp8 = mybir.dt.float8e4
    Exp = mybir.ActivationFunctionType.Exp
    DR = mybir.MatmulPerfMode.DoubleRow

    nc = bacc.Bacc("TRN2", target_bir_lowering=False, debug=False)

    qfa_d = nc.dram_tensor("qfa", [B, 64, L], fp8, kind="ExternalInput")
    qfb_d = nc.dram_tensor("qfb", [B, 128, L], fp8, kind="ExternalInput")
    kfa_d = nc.dram_tensor("kfa", [B, 128, L], fp8, kind="ExternalInput")
    kfb_d = nc.dram_tensor("kfb", [B, 64, L], fp8, kind="ExternalInput")
    const8_d = nc.dram_tensor("const8", [128, 512], fp8, kind="ExternalInput")
    v8_d = nc.dram_tensor("v8", [B, 128, 4, 2, 64], fp8, kind="ExternalInput")
    qres_d = nc.dram_tensor("qres", [128, 8, B, 64], f32, kind="ExternalInput")
    outt = nc.dram_tensor("outt", [B, 128, 8, 64], f32, kind="ExternalOutput")

    with tile.TileContext(nc) as tc:
        with (
            tc.tile_pool(name="persist", bufs=1) as persist,
            tc.tile_pool(name="work", bufs=2) as work,
            tc.tile_pool(name="expp", bufs=3) as expp,
            tc.tile_pool(name="outp", bufs=2) as outp,
        ):
            biasc = persist.tile([128, 1], f32)
            nc.gpsimd.memset(biasc, -4.0)
            z64a = persist.tile([64, 32], fp8)
            nc.gpsimd.memset(z64a, 0.0)
            warm = persist.tile([128, 1], f32)
            nc.scalar.activation(warm, biasc, Exp, bias=biasc)

            const_t = persist.tile([128, 512], fp8)
            nc.sync.dma_start(const_t, const8_d[:])
            tbl = const_t[0:64, 0:252].rearrange("c (t m) -> c t m", t=4)
            ones2 = const_t[:, 504:506].rearrange("p (i o) -> p i o", o=1)

            # per-batch operand tiles; batch-0 pieces issued first
            Qf = [persist.tile([128, 2, L], fp8, name=f"Qf{p}") for p in range(B)]
            Kf = [persist.tile([128, 2, L], fp8, name=f"Kf{p}") for p in range(B)]
            v8t = [persist.tile([128, 4, 2, 64], fp8, name=f"v8{p}") for p in range(B)]
            def load_batch(p, eng):
                # aug regions Qf[64:128, 0] / Kf[0:64, 1] are device-written;
                # batch 0's aug inputs ride SWDGE to dodge the serial HWDGE
                eng.dma_start(Qf[p][0:64, 0, :], qfa_d[p])
                eng.dma_start(Kf[p][:, 0, :], kfa_d[p])
                nc.sync.dma_start(Qf[p][:, 1, :], qfb_d[p])
                nc.sync.dma_start(Kf[p][64:128, 1, :], kfb_d[p])
                nc.sync.dma_start(v8t[p], v8_d[p])

            load_batch(0, nc.gpsimd)
            for p in range(1, B):
                load_batch(p, nc.sync)
            qres_t = persist.tile([128, 8, B, 64], f32)
            nc.sync.dma_start(qres_t, qres_d[:])

            # prologue aug(0) in two separate 2-bank tiles (q-side, k-side):
            # byte-interval dependency tracking is partition-blind, so a
            # shared tile would serialize k-strips behind q-copies; the
            # scoped pool frees its banks before the main pools open
            with tc.tile_pool(name="ps_aug0", bufs=2, space="PSUM") as ps_aug0:
                tq0 = ps_aug0.tile([64, L], f32, name="tq0")
                tk0 = ps_aug0.tile([64, L], f32, name="tk0")
                for t in range(4):
                    tile_, row0 = (tq0, 32 * t) if t < 2 else (tk0, 32 * (t - 2))
                    tile_w = tile_.rearrange("j (h w) -> j w h", w=32)
                    src = (Qf[0] if t < 2 else Kf[0])[0:64, 0, :]
                    src_w = src.rearrange("c (h w) -> c w h", w=32)
                    if t % 2 == 1:
                        for bank in range(2):
                            nc.tensor.matmul(
                                tile_[row0:row0 + 32,
                                      512 * bank:512 * bank + 1],
                                z64a, const_t[0:64, 0:1],
                                start=True, stop=True,
                                tile_position=(0, row0),
                                skip_group_check=True)
                    for g in range(32):
                        lhsT = tbl[:, t, 31 - g:63 - g]
                        if t % 2 == 0:
                            rhs = src[:, 32 * g:32 * g + 32]
                            out = tile_[row0:row0 + 32, 32 * g:32 * g + 32]
                            st = g in (0, 16)
                        else:
                            rhs = src_w[:, g, :]
                            out = tile_w[row0:row0 + 32, g, :]
                            st = False
                        nc.tensor.matmul(out, lhsT, rhs, start=st, stop=True,
                                         tile_position=(0, row0),
                                         skip_group_check=True)
                    if t == 1:
                        # one whole-side copy: S waits the LAST copy into its
                        # operand tile anyway, so fewer copies = fewer sem
                        # edges and queue gaps
                        nc.vector.tensor_copy(Qf[0][64:128, 0, :], tq0[:, :])
                    elif t == 3:
                        nc.vector.tensor_copy(Kf[0][0:64, 1, :], tk0[:, :])

            with (
                tc.tile_pool(name="ps_aug", bufs=1, space="PSUM") as ps_aug,
                tc.tile_pool(name="ps_s", bufs=2, space="PSUM") as ps_s,
                tc.tile_pool(name="ps_pv", bufs=1, space="PSUM") as ps_pv,
                tc.tile_pool(name="ps_den", bufs=1, space="PSUM") as ps_den,
            ):
                def emit_aug_strips(ps_a, ps_aw, p, ts):
                    for t in ts:
                        src = (Qf[p] if t < 2 else Kf[p])[0:64, 0, :]
                        src_w = src.rearrange("c (h w) -> c w h", w=32)
                        if t % 2 == 1:
                            for bank in range(2):
                                nc.tensor.matmul(
                                    ps_a[32 * t:32 * t + 32,
                                         512 * bank:512 * bank + 1],
                                    z64a, const_t[0:64, 0:1],
                                    start=True, stop=True,
                                    tile_position=(0, 32 * t),
                                    skip_group_check=True)
                        for g in range(32):
                            lhsT = tbl[:, t, 31 - g:63 - g]
                            if t % 2 == 0:
                                rhs = src[:, 32 * g:32 * g + 32]
                                out = ps_a[32 * t:32 * t + 32, 32 * g:32 * g + 32]
                                st = g in (0, 16)
                            else:
                                rhs = src_w[:, g, :]
                                out = ps_aw[32 * t:32 * t + 32, g, :]
                                st = False
                            nc.tensor.matmul(out, lhsT, rhs, start=st, stop=True,
                                             tile_position=(0, 32 * t),
                                             skip_group_check=True)

                aug_tiles = {}

                def emit_aug_step(p, step, act_assist=False):
                    if step == 0:
                        ps_a = ps_aug.tile([128, L], f32, tag="psa", name="psa")
                        aug_tiles[p] = (ps_a, ps_a.rearrange("j (h w) -> j w h",
                                                             w=32))
                    ps_a, ps_aw = aug_tiles[p]
                    emit_aug_strips(ps_a, ps_aw, p, (step,))
                    # ACT is idle before the first exp, so the prologue
                    # parallelizes the PSUM->SBUF copies across DVE + ACT
                    if step == 1:
                        nc.vector.tensor_copy(Qf[p][64:128, 0, 0:512],
                                              ps_a[0:64, 0:512])
                        (nc.scalar.copy if act_assist
                         else nc.vector.tensor_copy)(
                            Qf[p][64:128, 0, 512:L], ps_a[0:64, 512:L])
                    elif step == 3:
                        nc.vector.tensor_copy(Kf[p][0:64, 1, 0:128],
                                              ps_a[64:128, 0:128])
                        (nc.scalar.copy if act_assist
                         else nc.vector.tensor_copy)(
                            Kf[p][0:64, 1, 128:L], ps_a[64:128, 128:L])

                def emit_aug(p):
                    for step in range(4):
                        emit_aug_step(p, step)

                state = {}

                def emit_s_exp(p, kb):
                    st = state[p]
                    if kb % 2 == 0:
                        st["et2"].append(expp.tile([128, 2, L], fp8,
                                                   tag="et2", name="et2"))
                    et2 = st["et2"][kb // 2]
                    sp = ps_s.tile([128, L], f32, tag="sp", name="sp")
                    for ch in range(2):
                        cs = slice(512 * ch, 512 * (ch + 1))
                        nc.tensor.matmul(
                            sp[:, cs],
                            Kf[p][:, :, 128 * kb:128 * (kb + 1)],
                            Qf[p][:, :, cs],
                            start=True, stop=True, perf_mode=DR)
                    if p == B - 1 and kb == 7:
                        # split the very last exp so the first PV/normalize
                        # half overlaps the second half-exp (shorter drain)
                        for ch in range(2):
                            cs = slice(512 * ch, 512 * (ch + 1))
                            nc.scalar.activation(et2[:, 1, cs], sp[:, cs],
                                                 Exp, bias=biasc)
                    else:
                        nc.scalar.activation(et2[:, kb % 2, :], sp, Exp,
                                             bias=biasc)

                def emit_pv(p, kbp):
                    st = state[p]
                    et2 = st["et2"][kbp]
                    for qb in range(8):
                        lhsT = et2[:, :, 128 * qb:128 * (qb + 1)]
                        first = kbp == 0 and qb == 0
                        nc.tensor.matmul(st["pvt"][:, qb, :], lhsT,
                                         v8t[p][:, kbp, :, :],
                                         start=first, stop=(kbp == 3),
                                         perf_mode=DR, skip_group_check=True)
                        nc.tensor.matmul(st["den"][:, qb:qb + 1], lhsT, ones2,
                                         start=first, stop=(kbp == 3),
                                         perf_mode=DR, skip_group_check=True)

                def emit_norm(p, quarters=False):
                    # normalize + residual in qb-chunks so each output DMA
                    # overlaps the next chunk's vector work
                    st = state[p]
                    r = work.tile([128, 8], f32, tag="r", name="r")
                    nc.vector.reciprocal(r, st["den"])
                    ot = outp.tile([128, 8, 64], f32, tag="ot", name="ot")
                    nch = 4 if quarters else 2
                    w_ = 8 // nch
                    for hb in range(nch):
                        hs = slice(w_ * hb, w_ * hb + w_)
                        otm = work.tile([128, w_, 64], f32, tag="otm",
                                        name="otm", bufs=2)
                        nc.vector.tensor_mul(
                            otm, st["pvt"][:, hs, :],
                            r[:, hs, None].to_broadcast((128, w_, 64)))
                        (nc.gpsimd if hb % 2 else nc.vector).tensor_add(
                            ot[:, hs, :], otm, qres_t[:, hs, p, :])
                        nc.sync.dma_start(outt[p, :, hs, :], ot[:, hs, :])

                # software pipeline: PV(p, m) emits three S/exp slots
                # after exp(p, 2m+1) and the next batch's aug strips spread
                # over kb 2..5, so the in-order PE queue never blocks the
                # exp stream; normalize(p) slides into batch p+1
                stream = [(p, kb) for p in range(B) for kb in range(8)]
                for idx, (p, kb) in enumerate(stream):
                    if kb == 0:
                        state[p] = dict(
                            pvt=ps_pv.tile([128, 8, 64], f32, tag="pvt",
                                           name="pvt"),
                            den=ps_den.tile([128, 8], f32, tag="den",
                                            name="den"),
                            et2=[])
                    emit_s_exp(p, kb)
                    aug0 = 2
                    if aug0 <= kb <= aug0 + 3 and p + 1 < B:
                        emit_aug_step(p + 1, kb - aug0)
                    due = idx - 3
                    if due >= 0:
                        dp, dkb = stream[due]
                        if dkb % 2 == 1:
                            emit_pv(dp, dkb // 2)
                            if dkb == 7:
                                emit_norm(dp)
                # flush: PV2 then the last k-pair + normalize in
                # q-halves pipelined against the split final exp
                lp = B - 1
                emit_pv(lp, 2)
                st = state[lp]
                et2 = st["et2"][3]
                r = work.tile([128, 8], f32, tag="r", name="r")
                ot = outp.tile([128, 8, 64], f32, tag="ot", name="ot")
                for half in range(2):
                    for qb in range(4 * half, 4 * half + 4):
                        lhsT = et2[:, :, 128 * qb:128 * (qb + 1)]
                        nc.tensor.matmul(st["pvt"][:, qb, :], lhsT,
                                         v8t[lp][:, 3, :, :],
                                         start=False, stop=True,
                                         perf_mode=DR, skip_group_check=True)
                        nc.tensor.matmul(st["den"][:, qb:qb + 1], lhsT, ones2,
                                         start=False, stop=True,
                                         perf_mode=DR, skip_group_check=True)
                    hs4 = slice(4 * half, 4 * half + 4)
                    nc.vector.reciprocal(r[:, hs4], st["den"][:, hs4])
                    for sub in range(2):
                        hs = slice(4 * half + 2 * sub, 4 * half + 2 * sub + 2)
                        otm = work.tile([128, 2, 64], f32, tag="otm",
                                        name="otm", bufs=2)
                        nc.vector.tensor_mul(
                            otm, st["pvt"][:, hs, :],
                            r[:, hs, None].to_broadcast((128, 2, 64)))
                        # first sub's add on Pool overlaps the second sub's
                        # DVE mul; the critical last add stays on fast DVE
                        (nc.vector if sub else nc.gpsimd).tensor_add(
                            ot[:, hs, :], otm, qres_t[:, hs, lp, :])
                    nc.sync.dma_start(outt[lp, :, hs4, :], ot[:, hs4, :])

    nc.compile()
    return nc


def _split_c(x):
    # [64, ...] -> [32, 2, ...] with c = 32*i + ci
    return np.ascontiguousarray(
        x.reshape(2, 32, *x.shape[1:]).transpose(1, 0, *range(2, x.ndim + 1)))


def kernel(query, key_input, value, rel_h_q, rel_w_q, rel_h_k, rel_w_k):
    import ml_dtypes
    from concourse.bass_utils import run_bass_kernel_spmd

    f8 = ml_dtypes.float8_e4m3
    query = np.asarray(query, np.float32)
    key_input = np.asarray(key_input, np.float32)
    value = np.asarray(value, np.float32)

    if "nc" not in _CACHED:
        _CACHED["nc"] = _build_nc()
    nc = _CACHED["nc"]

    ll = np.arange(L)
    oh_h = (ll // 32 == np.arange(32)[:, None]).astype(np.float32)  # [32, L]
    oh_w = (ll % 32 == np.arange(32)[:, None]).astype(np.float32)

    # tables [4(t), 64(c), 63(m)] -> const8 rows 0:64; q-side tables x8
    tables = np.stack([
        np.asarray(rel_h_q, np.float32)[::-1].T * 8.0,
        np.asarray(rel_w_q, np.float32)[::-1].T * 8.0,
        np.asarray(rel_h_k, np.float32).T,
        np.asarray(rel_w_k, np.float32).T,
    ], 0)
    const8 = np.zeros((128, 512), np.float32)
    const8[0:64, 0:252] = tables.transpose(1, 0, 2).reshape(64, 252)
    const8[:, 504:506] = 1.0
    const8 = const8.astype(f8)

    z64 = np.zeros((64, L), np.float32)

    in_maps = []
    for n in range(NCORES):
        q = query[:, n]           # [B, L, C]
        k = key_input[:, n]
        v = value[:, n]
        qT = q.transpose(2, 0, 1)  # [C, B, L]
        kT = k.transpose(2, 0, 1)
        # qf[p]: [128, 2, L]: pair0 = [Q^T/8 ; qterm placeholder]
        #                     pair1 = [onehot_h(q); onehot_w(q); zeros]
        qfa = np.ascontiguousarray(qT.transpose(1, 0, 2) / 8.0).astype(f8)
        qfb1 = np.concatenate([oh_h, oh_w, z64], 0)  # [128, L]
        qfb = np.ascontiguousarray(
            np.broadcast_to(qfb1[None], (B, 128, L))).astype(f8)
        kfa = np.ascontiguousarray(np.concatenate(
            [kT.transpose(1, 0, 2),
             np.broadcast_to(oh_h[None], (B, 32, L)),
             np.broadcast_to(oh_w[None], (B, 32, L))], 1)).astype(f8)
        kfb = np.zeros((B, 64, L), f8)
        # v8[p]: [128, 4(kbp), 2(i), 64]; k = (2*kbp + i)*128 + kp
        v8 = np.ascontiguousarray(
            v.reshape(B, 4, 2, 128, 64).transpose(0, 3, 1, 2, 4)).astype(f8)
        qres = np.ascontiguousarray(
            q.reshape(B, 8, 128, 64).transpose(2, 1, 0, 3)).astype(np.float32)
        in_maps.append(dict(qfa=qfa, qfb=qfb, kfa=kfa, kfb=kfb,
                            const8=const8, v8=v8, qres=qres))

    res = run_bass_kernel_spmd(
        nc, in_maps, core_ids=list(range(NCORES)),
        trace=bool(int(os.environ.get("KERNEL_TRACE", "0"))),
    )
    _CACHED["last_result"] = res

    # outt: [B, 128, 8, 64] -> out[b, n, qb*128+qp, c]
    out = np.stack([r["outt"] for r in res.results], axis=1)  # [B, NH, 128, 8, 64]
    out = out.transpose(0, 1, 3, 2, 4).reshape(B, NH, L, C)
    return np.ascontiguousarray(out).astype(np.float32)

